# revision 18
# baseline (speedup 1.0000x reference)
"""BlockLSTM Trainium2 kernel.

Problem: B=64, S=1024, IN=1024, H=1024 LSTM (4 gates, swapped-carry variant):
    g = x_t @ Wx + bx + h @ Wh + bh          # [4,B,H], gates (i, f, gg, o)
    c_new = sig(g_f)*c + sig(g_i)*tanh(g_gg); h_new = sig(g_o)*tanh(c_new)
    carry' = (c_new, h_new)   (swapped: next-h <- c_new, next-c <- h_new)
    per-step output = sig(g_o)
Returns (outs [B,S,H], h_fin [B,H], c_fin [B,H]).

Strategy: data-parallel over batch (8 sequences/core, zero communication).
Phase 1 precomputes XW = x@Wx + (bx+bh) at full PE efficiency; phase 2 runs
the recurrence with 4-way column-tiled matmuls streaming Wh from SBUF and
partition-packed elementwise ops.

Per-step gate placement in PSUM (partition base: content):
  round 0 bank: 0:8 gg[:512], 32:40 gg[512:], 64:72 f[:512], 96:104 f[512:]
  round 1 bank: 0:8 i[:512],  32:40 i[512:],  64:72 o[:512], 96:104 o[512:]

All tile pools stay open for the whole program (no release/realloc reuse);
the Wx SBUF tiles are overwritten in place with Wh between the phases.
"""

import os
import sys
from contextlib import ExitStack

import numpy as np

if "/opt/trn_rl_repo" not in sys.path:
    sys.path.insert(0, "/opt/trn_rl_repo")

import concourse.bass as bass  # noqa: E402
import concourse.mybir as mybir  # noqa: E402
import concourse.tile as tile  # noqa: E402
from concourse import bacc  # noqa: E402
from concourse.bass_utils import run_bass_kernel_spmd  # noqa: E402

B, S_FULL, IN, H = 64, 1024, 1024, 1024
NCORES = 8
BC = B // NCORES  # 8 sequences per core
G4 = 4 * H  # 4096 gate columns
NCH = G4 // 512  # 8 chunks of 512 gate-cols
FP = mybir.dt.float32
ACT = mybir.ActivationFunctionType

# chunk c (512 gate-cols) belongs to gate c//2;  gates: 0=i 1=f 2=gg 3=o
ROUND_CHUNKS = [
    [4, 5, 2, 3],  # round 0 groups 0..3: gg0 gg1 f0 f1
    [0, 1, 6, 7],  # round 1 groups 0..3: i0 i1 o0 o1
]
CHUNK_RG = {}
for _r, _cs in enumerate(ROUND_CHUNKS):
    for _g, _c in enumerate(_cs):
        CHUNK_RG[_c] = (_r, _g)


def build_tile_kernel(tc, s_len, aps):
    nc = tc.nc
    n_steps = int(os.environ.get("LSTM_STEPS", s_len))
    skip_p1 = bool(int(os.environ.get("LSTM_SKIP_P1", "0")))
    no_tr = bool(int(os.environ.get("LSTM_NO_TR", "0")))
    no_sel = bool(int(os.environ.get("LSTM_NO_SEL", "0")))
    no_out = bool(int(os.environ.get("LSTM_NO_OUT", "0")))
    ntok = BC * s_len
    n_mt = ntok // 128
    tch = s_len // 128

    (x2d, h0, c0, wx, wh, biasrow, onesrow, ident, sel32,
     outs, hfin, cfin, xw) = aps

    ctx = ExitStack()
    const_pool = ctx.enter_context(tc.tile_pool(name="const", bufs=1))
    ident_sb = const_pool.tile([128, 128], FP, tag="id")
    nc.sync.dma_start(ident_sb[:], ident[:])
    ones_sb = const_pool.tile([128, 128], FP, tag="ones")
    nc.sync.dma_start(ones_sb[:], onesrow[:])
    bias_sb = const_pool.tile([128, G4], FP, tag="bias")
    nc.sync.dma_start(bias_sb[:], biasrow[:])
    sel_sb = const_pool.tile([32, 128], FP, tag="sel")
    nc.sync.dma_start(sel_sb[:], sel32[:])

    # Weight tiles: hold Wx during phase 1, overwritten with Wh after.
    wpool = ctx.enter_context(tc.tile_pool(name="wp", bufs=1))
    w_sb = []
    for k in range(8):
        t = wpool.tile([128, G4], FP, tag=f"w{k}")
        nc.sync.dma_start(t[:], wx[k * 128:(k + 1) * 128, :])
        w_sb.append(t)

    # phase-1 pools (stay allocated, idle during phase 2)
    xinp = ctx.enter_context(tc.tile_pool(name="xin", bufs=2))
    xtp = ctx.enter_context(tc.tile_pool(name="xt", bufs=2))
    stgp = ctx.enter_context(tc.tile_pool(name="stg", bufs=2))
    # psum pools shared by both phases (8 banks total)
    psA = ctx.enter_context(tc.tile_pool(name="psA", bufs=2, space="PSUM"))
    psB = ctx.enter_context(tc.tile_pool(name="psB", bufs=2, space="PSUM"))
    ptp = ctx.enter_context(tc.tile_pool(name="ptp", bufs=2, space="PSUM"))
    auxp = ctx.enter_context(tc.tile_pool(name="auxp", bufs=1, space="PSUM"))
    # phase-2 pools
    xld = ctx.enter_context(tc.tile_pool(name="xld", bufs=4))
    actp = ctx.enter_context(tc.tile_pool(name="actp", bufs=2))
    dvep = ctx.enter_context(tc.tile_pool(name="dvep", bufs=2))
    stp = ctx.enter_context(tc.tile_pool(name="stp", bufs=2))

    # ================= Phase 1: XW = x @ Wx + bias =================
    for m in range(0 if skip_p1 else n_mt):
        bidx, t0 = m // tch, (m % tch) * 128
        xin = xinp.tile([128, IN], FP, tag="xin")
        nc.sync.dma_start(xin[:], x2d[m * 128:(m + 1) * 128, :])
        xt = xtp.tile([128, IN], FP, tag="xt")
        for k in range(8):
            ptx = ptp.tile([128, 128], FP, tag="pt")
            nc.tensor.transpose(
                ptx[:], xin[:, k * 128:(k + 1) * 128], ident_sb[:]
            )
            nc.scalar.copy(xt[:, k * 128:(k + 1) * 128], ptx[:])
        for n in range(NCH):
            pool = psA if n % 2 == 0 else psB
            ps = pool.tile([128, 512], FP, tag="ps")
            nc.tensor.matmul(
                ps[:], ones_sb[:, 0:128], bias_sb[:, n * 512:(n + 1) * 512],
                start=True, stop=False, skip_group_check=True,
            )
            for k in range(8):
                nc.tensor.matmul(
                    ps[:],
                    xt[:, k * 128:(k + 1) * 128],
                    w_sb[k][:, n * 512:(n + 1) * 512],
                    start=False, stop=(k == 7), skip_group_check=True,
                )
            stg = stgp.tile([128, 512], FP, tag="stg")
            nc.scalar.copy(stg[:], ps[:])
            r, g = CHUNK_RG[n]
            nc.sync.dma_start(xw[t0:t0 + 128, r, 8 * g + bidx, :], stg[:])

    # ============ overwrite Wx tiles with Wh (WAR-tracked) ============
    for k in range(8):
        nc.sync.dma_start(w_sb[k][:], wh[k * 128:(k + 1) * 128, :])

    # ================= Phase 2: recurrence =================
    # Initial c-state at partitions {64:72}=h<512, {96:104}=h>=512.
    ct = stp.tile([128, 512], FP, tag="ct")
    nc.gpsimd.memset(ct[:], 0.0)
    nc.sync.dma_start(ct[64:72, :], c0[:, 0:512])
    nc.sync.dma_start(ct[96:104, :], c0[:, 512:1024])
    # Initial h, staged then transposed into hT [128, 64] (h-chunk j at cols 8j).
    h0s = dvep.tile([128, 1024], FP, tag="cn")
    nc.sync.dma_start(h0s[0:8, :], h0[:, :])
    pt0 = ptp.tile([128, 128], FP, tag="pt")
    for j in range(8):
        nc.tensor.transpose(
            pt0[:, 8 * j:8 * j + 8],
            h0s[0:8, 128 * j:128 * j + 128],
            ident_sb[0:8, 0:8],
        )
    ht = stp.tile([128, BC * 8], FP, tag="ht")
    nc.scalar.copy(ht[:], pt0[:, 0:BC * 8])

    cn = None
    for t in range(n_steps):
        # ---- xw tiles for both rounds (prefetchable) ----
        xs = []
        if not no_sel:
            for r in range(2):
                xt_ = xld.tile([32, 512], FP, tag="xs")
                nc.sync.dma_start(xt_[:], xw[t, r])
                xs.append(xt_)

        # ---- matmuls: psum = xw(+bias) + h @ Wh ----
        ps = []
        for r in range(2):
            pool = psA if r == 0 else psB
            p = pool.tile([128, 512], FP, tag="ps")
            ps.append(p)
            if not no_sel:
                nc.tensor.matmul(
                    p[0:104, :], sel_sb[:, 0:104], xs[r][:],
                    start=True, stop=False, skip_group_check=True,
                )
            for k in range(8):
                for g in range(4):
                    chunk = ROUND_CHUNKS[r][g]
                    nc.tensor.matmul(
                        p[32 * g:32 * g + 8, :],
                        ht[:, 8 * k:8 * k + 8],
                        w_sb[k][:, chunk * 512:(chunk + 1) * 512],
                        start=(no_sel and k == 0), stop=(k == 7),
                        tile_position=(0, 32 * g),
                        skip_group_check=True,
                    )

        # ---- activations ----
        a0 = actp.tile([128, 512], FP, tag="a0")  # tanh(gg) @0:40, sig(f) @64:104
        nc.scalar.activation(a0[0:40, :], ps[0][0:40, :], ACT.Tanh)
        nc.scalar.activation(a0[64:104, :], ps[0][64:104, :], ACT.Sigmoid)
        a1 = actp.tile([128, 512], FP, tag="a1")  # sig(i) @0:40, sig(o) @64:104
        nc.scalar.activation(a1[0:104, :], ps[1][0:104, :], ACT.Sigmoid)

        # ---- elementwise: c_new = f*c + i*gg ----
        fc = auxp.tile([128, 512], FP, tag="fc")
        nc.vector.tensor_mul(fc[64:72, :], a0[64:72, :], ct[64:72, :])
        nc.vector.tensor_mul(fc[96:104, :], a0[96:104, :], ct[96:104, :])
        ig = dvep.tile([128, 512], FP, tag="ig")
        nc.vector.tensor_mul(ig[0:40, :], a1[0:40, :], a0[0:40, :])
        cn = dvep.tile([128, 1024], FP, tag="cn")
        nc.vector.tensor_add(cn[0:8, 0:512], ig[0:8, :], fc[64:72, :])
        cn1 = dvep.tile([128, 512], FP, tag="cn1")
        nc.vector.tensor_add(cn1[32:40, :], ig[32:40, :], fc[96:104, :])
        nc.vector.tensor_copy(cn[0:8, 512:1024], cn1[32:40, :])

        # ---- next h (= c_new) transposed via PE ----
        ht = stp.tile([128, BC * 8], FP, tag="ht")
        if no_tr:
            nc.scalar.copy(ht[:], a0[:, 0:BC * 8])
        else:
            pt = ptp.tile([128, 128], FP, tag="pt")
            for j in range(8):
                nc.tensor.transpose(
                    pt[:, 8 * j:8 * j + 8],
                    cn[0:8, 128 * j:128 * j + 128],
                    ident_sb[0:8, 0:8],
                )
            if bool(int(os.environ.get("LSTM_HT_DVE", "0"))):
                nc.vector.tensor_copy(ht[:], pt[:, 0:BC * 8])
            else:
                nc.scalar.copy(ht[:], pt[:, 0:BC * 8])

        # ---- next c (= h_new = o * tanh(c_new)) ----
        tcn = auxp.tile([128, 512], FP, tag="tcn")
        nc.scalar.activation(tcn[64:72, :], cn[0:8, 0:512], ACT.Tanh)
        nc.scalar.activation(tcn[96:104, :], cn[0:8, 512:1024], ACT.Tanh)
        ct = stp.tile([128, 512], FP, tag="ct")
        nc.vector.tensor_mul(ct[64:72, :], a1[64:72, :], tcn[64:72, :])
        nc.vector.tensor_mul(ct[96:104, :], a1[96:104, :], tcn[96:104, :])

        # ---- store output o_t ----
        if not no_out:
            nc.sync.dma_start(outs[0:BC, t, 0:512], a1[64:72, :])
            nc.sync.dma_start(outs[0:BC, t, 512:1024], a1[96:104, :])

    # final states: h_fin = c_new(last), c_fin = h_new(last)
    if cn is None:
        cn = ct
    nc.sync.dma_start(hfin[:, :], cn[0:8, 0:1024])
    nc.sync.dma_start(cfin[:, 0:512], ct[64:72, :])
    nc.sync.dma_start(cfin[:, 512:1024], ct[96:104, :])
    ctx.close()


def build_program(s_len: int, n_devices: int):
    nc = bacc.Bacc(
        "TRN2", target_bir_lowering=False, debug=False, num_devices=n_devices
    )
    ntok = BC * s_len
    aps = (
        nc.dram_tensor("x2d", [ntok, IN], FP, kind="ExternalInput").ap(),
        nc.dram_tensor("h0", [BC, H], FP, kind="ExternalInput").ap(),
        nc.dram_tensor("c0", [BC, H], FP, kind="ExternalInput").ap(),
        nc.dram_tensor("wx", [IN, G4], FP, kind="ExternalInput").ap(),
        nc.dram_tensor("wh", [H, G4], FP, kind="ExternalInput").ap(),
        nc.dram_tensor("biasrow", [128, G4], FP, kind="ExternalInput").ap(),
        nc.dram_tensor("onesrow", [128, 128], FP, kind="ExternalInput").ap(),
        nc.dram_tensor("ident", [128, 128], FP, kind="ExternalInput").ap(),
        nc.dram_tensor("sel32", [32, 128], FP, kind="ExternalInput").ap(),
        nc.dram_tensor("outs", [BC, s_len, H], FP, kind="ExternalOutput").ap(),
        nc.dram_tensor("hfin", [BC, H], FP, kind="ExternalOutput").ap(),
        nc.dram_tensor("cfin", [BC, H], FP, kind="ExternalOutput").ap(),
        nc.dram_tensor("xw", [s_len, 2, 32, 512], FP, kind="Internal").ap(),
    )
    with tile.TileContext(nc) as tc:
        build_tile_kernel(tc, s_len, aps)
    nc.compile()
    return nc


_CACHE = {}


def _get_program(s_len, n_devices):
    key = (s_len, n_devices)
    if key not in _CACHE:
        _CACHE[key] = build_program(s_len, n_devices)
    return _CACHE[key]


def host_inputs(x, h0, c0, Wx, bx, Wh, bh):
    """Per-core input maps (list of dicts keyed by dram tensor name)."""
    s_len = x.shape[1]
    x = np.asarray(x, np.float32)
    h0 = np.asarray(h0, np.float32)
    c0 = np.asarray(c0, np.float32)
    wx_m = np.ascontiguousarray(
        np.asarray(Wx, np.float32).transpose(1, 0, 2).reshape(IN, G4)
    )
    wh_m = np.ascontiguousarray(
        np.asarray(Wh, np.float32).transpose(1, 0, 2).reshape(H, G4)
    )
    bias = (np.asarray(bx, np.float32) + np.asarray(bh, np.float32)).reshape(G4)
    biasrow = np.zeros((128, G4), np.float32)
    biasrow[0] = bias
    onesrow = np.zeros((128, 128), np.float32)
    onesrow[0] = 1.0
    ident = np.eye(128, dtype=np.float32)
    sel32 = np.zeros((32, 128), np.float32)
    for p in range(32):
        sel32[p, 32 * (p // 8) + (p % 8)] = 1.0

    in_maps = []
    for c in range(NCORES):
        sl = slice(c * BC, (c + 1) * BC)
        in_maps.append({
            "x2d": np.ascontiguousarray(x[sl].reshape(BC * s_len, IN)),
            "h0": np.ascontiguousarray(h0[sl]),
            "c0": np.ascontiguousarray(c0[sl]),
            "wx": wx_m,
            "wh": wh_m,
            "biasrow": biasrow,
            "onesrow": onesrow,
            "ident": ident,
            "sel32": sel32,
        })
    return in_maps


LAST_EXEC_NS = None


def kernel(x, h0, c0, Wx, bx, Wh, bh):
    global LAST_EXEC_NS
    s_len = x.shape[1]
    nc = _get_program(s_len, NCORES)
    in_maps = host_inputs(x, h0, c0, Wx, bx, Wh, bh)
    trace = bool(int(os.environ.get("LSTM_TRACE", "0")))
    res = run_bass_kernel_spmd(nc, in_maps, list(range(NCORES)), trace=trace)
    LAST_EXEC_NS = res.exec_time_ns
    outs = np.empty((B, s_len, H), np.float32)
    h_fin = np.empty((B, H), np.float32)
    c_fin = np.empty((B, H), np.float32)
    for c in range(NCORES):
        sl = slice(c * BC, (c + 1) * BC)
        outs[sl] = res.results[c]["outs"]
        h_fin[sl] = res.results[c]["hfin"]
        c_fin[sl] = res.results[c]["cfin"]
    return outs, h_fin, c_fin


def bench_exec(x, h0, c0, Wx, bx, Wh, bh, iters=4):
    """Time the on-device execution with device-resident inputs.

    Returns (best_seconds, results_list) where results_list matches
    run_bass_kernel_spmd's per-core output dicts.
    """
    import time

    import jax
    from jax.sharding import Mesh, NamedSharding, PartitionSpec
    from jax.experimental.shard_map import shard_map

    from concourse import bass2jax, mybir as _mybir

    bass2jax.install_neuronx_cc_hook()
    s_len = x.shape[1]
    nc = _get_program(s_len, NCORES)
    in_maps = host_inputs(x, h0, c0, Wx, bx, Wh, bh)

    part_name = (nc.partition_id_tensor.name if nc.partition_id_tensor
                 else None)
    in_names, out_names, out_avals, zero_outs = [], [], [], []
    for alloc in nc.m.functions[0].allocations:
        if not isinstance(alloc, _mybir.MemoryLocationSet):
            continue
        name = alloc.memorylocations[0].name
        if alloc.kind == "ExternalInput":
            if name != part_name:
                in_names.append(name)
        elif alloc.kind == "ExternalOutput":
            shape = tuple(alloc.tensor_shape)
            dtype = _mybir.dt.np(alloc.dtype)
            out_names.append(name)
            out_avals.append(jax.core.ShapedArray(shape, dtype))
            zero_outs.append(np.zeros(shape, dtype))
    n_params = len(in_names)
    n_outs = len(out_avals)
    all_in_names = in_names + out_names
    if part_name is not None:
        all_in_names = all_in_names + [part_name]

    def _body(*args):
        operands = list(args)
        if part_name is not None:
            operands.append(bass2jax.partition_id_tensor())
        outs = bass2jax._bass_exec_p.bind(
            *operands,
            out_avals=tuple(out_avals),
            in_names=tuple(all_in_names),
            out_names=tuple(out_names),
            lowering_input_output_aliases=(),
            sim_require_finite=True,
            sim_require_nnan=True,
            nc=nc,
        )
        return tuple(outs)

    devices = jax.devices()[:NCORES]
    mesh = Mesh(np.asarray(devices), ("core",))
    spec = NamedSharding(mesh, PartitionSpec("core"))
    donate = tuple(range(n_params, n_params + n_outs))
    sharded = jax.jit(
        shard_map(_body, mesh=mesh,
                  in_specs=(PartitionSpec("core"),) * (n_params + n_outs),
                  out_specs=(PartitionSpec("core"),) * n_outs,
                  check_rep=False),
        donate_argnums=donate, keep_unused=True,
    )
    concat_in = [
        np.concatenate([np.asarray(in_maps[c][nm]) for c in range(NCORES)], axis=0)
        for nm in in_names
    ]
    dev_in = [jax.device_put(a, spec) for a in concat_in]
    jax.block_until_ready(dev_in)

    best, outs_np = None, None
    for _ in range(iters):
        zeros_dev = [
            jax.device_put(np.zeros((NCORES * z.shape[0], *z.shape[1:]), z.dtype), spec)
            for z in zero_outs
        ]
        jax.block_until_ready(zeros_dev)
        t0 = time.perf_counter()
        out_arrs = sharded(*dev_in, *zeros_dev)
        jax.block_until_ready(out_arrs)
        dt = time.perf_counter() - t0
        if best is None or dt < best:
            best = dt
            outs_np = [np.asarray(o) for o in out_arrs]
    results = [
        {nm: outs_np[i].reshape(NCORES, *out_avals[i].shape)[c]
         for i, nm in enumerate(out_names)}
        for c in range(NCORES)
    ]
    return best, results
